# revision 19
# baseline (speedup 1.0000x reference)
"""CQAttention Bass/Tile kernel for Trainium2, 8 NeuronCores, batch-parallel.

Math (per batch, all derived from the reference):
  ct = c^T (Lc,d), qt = q^T (Lq,d)
  s[i,j] = cq[i,j] + r_i + t_j (+b),  cq = (c*w_cq)^T q,  r = w_c^T c, t = w_q^T q
  s1 = softmax_j(s*cm_i + (1-cm_i)*-1e30)  -> row consts (r_i, b) cancel:
       unmasked row: softmax_j(cq+t); masked row: uniform 1/Lq
  s2 = softmax_i(s*qm_j + ...)             -> col consts (t_j, b) cancel:
       unmasked col: softmax_i(cq+r); masked col: uniform 1/Lc
  A = s1 @ qt ; B = s1 @ (s2^T @ ct)
  out = [ct, A, ct*A, ct*B]^T  (4d, Lc)  -- assembled in (d, Lc) layout.

Implementation choices:
  - E1^T = exp(cq^T + t_j) in (Lq-part, Lc-free) layout (fp32), fp32r matmuls.
  - s1^T = E1^T * Gb, Gb = broadcast of gamma_i = cm_i/rs_i (bf16), built by
    K=1 matmuls; masked-row uniform term handled as rank-1 (qsum x u) matmuls
    accumulated into the A/B psums, u_i = (1-cm_i)/Lq.
  - F = exp(cq + r_i) in (Lc-part, Lq-free) layout (bf16) feeds s2tc = s2^T@ct
    with per-partition (qm_j/cs_j) scaling + rank-1 (u2 x csum) masked fix.
  - Per-row scalars live as (128, n) column-chunked tiles (rs, cm, gamma...).
"""

import numpy as np

import concourse.bass as bass
import concourse.mybir as mybir
import concourse.tile as tile
from concourse import bacc
import ml_dtypes
from concourse.bass_utils import run_bass_kernel_spmd

F32 = mybir.dt.float32
F32R = mybir.dt.float32r
BF16 = mybir.dt.bfloat16
I32 = mybir.dt.int32
EXP = mybir.ActivationFunctionType.Exp
COPY = mybir.ActivationFunctionType.Copy
MUL = mybir.AluOpType.mult
ADD = mybir.AluOpType.add

B, D, LC, LQ = 32, 128, 2048, 256
NCORES = 8
BPC = B // NCORES  # batches per core
NLC = LC // 128    # 16 Lc chunks of 128
NJC = LQ // 128    # 2 Lq chunks of 128
NT = LC // 512     # 4 Lc tiles of 512


def r32(ap):
    return ap.bitcast(F32R)


def build_nc():
    nc = bacc.Bacc(None, target_bir_lowering=False, debug=False)

    c_d = nc.declare_dram_parameter("c", [BPC, D, LC], F32, isOutput=False)
    cm_d = nc.declare_dram_parameter("c_mask", [BPC, LC], I32, isOutput=False)
    q_d = nc.declare_dram_parameter("q", [BPC, D, LQ], F32, isOutput=False)
    qm_d = nc.declare_dram_parameter("q_mask", [BPC, LQ], I32, isOutput=False)
    w_d = nc.declare_dram_parameter("w", [3 * D], F32, isOutput=False)
    id_d = nc.declare_dram_parameter("ident", [128, 128], BF16, isOutput=False)
    out_d = nc.declare_dram_parameter("out", [BPC, 4 * D, LC], F32, isOutput=True)

    with tile.TileContext(nc) as tc:
        with (
            tc.tile_pool(name="const", bufs=1) as cst,
            tc.tile_pool(name="io", bufs=2) as io,
            tc.tile_pool(name="big", bufs=2) as big,
            tc.tile_pool(name="sml", bufs=2) as sml,
            # PSUM: 8 banks total. Tag budget (bufs x 1 bank each):
            #   sp=2 (S/S^T matmul), gb=1, a=2, b=1, misc=2  => 8
            tc.tile_pool(name="ps", bufs=1, space=bass.MemorySpace.PSUM) as ps,
        ):
            # ---- constants ----
            ident = cst.tile([128, 128], BF16)
            nc.sync.dma_start(out=ident, in_=id_d[:, :])
            ones_col_f = cst.tile([128, 1], F32)
            nc.vector.memset(ones_col_f, 1.0)
            ones_col_b = cst.tile([128, 1], BF16)
            nc.vector.memset(ones_col_b, 1.0)
            ones_row_b = cst.tile([1, 128], BF16)
            nc.vector.memset(ones_row_b, 1.0)
            wq_t = cst.tile([128, 1], F32)
            nc.sync.dma_start(out=wq_t, in_=w_d[0:D].rearrange("(p o) -> p o", o=1))
            wc_t = cst.tile([128, 1], F32)
            nc.sync.dma_start(out=wc_t, in_=w_d[D:2 * D].rearrange("(p o) -> p o", o=1))
            wcq_t = cst.tile([128, 1], F32)
            nc.sync.dma_start(out=wcq_t, in_=w_d[2 * D:3 * D].rearrange("(p o) -> p o", o=1))

            for b in range(BPC):
                # ---- loads ----
                c_t = io.tile([128, LC], F32, tag="c_t")
                nc.sync.dma_start(out=c_t, in_=c_d[b])
                q_t = io.tile([128, LQ], F32, tag="q_t")
                nc.sync.dma_start(out=q_t, in_=q_d[b])
                cm_i = sml.tile([128, NLC], I32, tag="cm_i")
                nc.sync.dma_start(out=cm_i, in_=cm_d[b].rearrange("(ii p) -> p ii", p=128))
                qm_i = sml.tile([128, NJC], I32, tag="qm_i")
                nc.sync.dma_start(out=qm_i, in_=qm_d[b].rearrange("(jj p) -> p jj", p=128))

                cm_f = sml.tile([128, NLC], F32, tag="cm_f")
                nc.vector.tensor_copy(cm_f, cm_i)
                qm_f = sml.tile([128, NJC], F32, tag="qm_f")
                nc.vector.tensor_copy(qm_f, qm_i)

                # ---- derived operands (all-bf16 matmul plan) ----
                # cq = c^T (q*w_cq): the w_cq scale rides the q operand so the
                # plain bf16 cb serves both S-matmuls; w_c rides as an extra
                # rhs column so r_i falls out of the S-matmul for free.
                qw_t = sml.tile([128, LQ + 1], BF16, tag="qw_t")
                nc.vector.tensor_scalar_mul(qw_t[:, 0:LQ], q_t, wcq_t[:, 0:1])
                nc.vector.tensor_copy(qw_t[:, LQ:LQ + 1], wc_t)
                cb_t = big.tile([128, LC], BF16, tag="cb_t")  # bf16 c + row sums
                csum_t = sml.tile([128, 1], F32, tag="csum_t")
                nc.scalar.activation(cb_t, c_t, COPY, accum_out=csum_t)
                qb_t = sml.tile([128, LQ], BF16, tag="qb_t")
                qsum_t = sml.tile([128, 1], F32, tag="qsum_t")
                nc.scalar.activation(qb_t, q_t, COPY, accum_out=qsum_t)
                wq_b = sml.tile([128, 1], BF16, tag="wq_b")
                nc.vector.tensor_copy(wq_b, wq_t)

                # t (128,2) via ap=1 bf16 matmuls
                t_ps = ps.tile([128, NJC], F32, tag="misc", bufs=2, name="t_ps")
                for jc in range(NJC):
                    nc.tensor.matmul(
                        t_ps[:, jc:jc + 1], qb_t[:, jc * 128:(jc + 1) * 128],
                        wq_b, start=(jc == 0), stop=(jc == NJC - 1))
                t_sb = sml.tile([128, NJC], F32, tag="t_sb")
                nc.vector.tensor_copy(t_sb, t_ps)
                r_sb = sml.tile([128, NLC], F32, tag="r_sb")

                # ---- E1^T = exp(cq^T + t_j), (Lq-part, Lc-free) bf16 ----
                e1_t = big.tile([128, NJC, LC], BF16, tag="e1_t")
                for jc in range(NJC):
                    for n in range(NT):
                        st_ps = ps.tile([128, 512], F32, tag="sp", bufs=2, name="st_ps")
                        nc.tensor.matmul(
                            st_ps, qw_t[:, jc * 128:(jc + 1) * 128],
                            cb_t[:, n * 512:(n + 1) * 512], start=True, stop=True)
                        nc.scalar.activation(
                            e1_t[:, jc, n * 512:(n + 1) * 512], st_ps, EXP,
                            bias=t_sb[:, jc:jc + 1])

                # row sums rs_i as (128,16)
                rs_ps = ps.tile([128, NLC], F32, tag="misc", bufs=2, name="rs_ps")
                for ii in range(NLC):
                    for jc in range(NJC):
                        nc.tensor.matmul(
                            rs_ps[:, ii:ii + 1], e1_t[:, jc, ii * 128:(ii + 1) * 128],
                            ones_col_b, start=(ii == 0 and jc == 0),
                            stop=(ii == NLC - 1 and jc == NJC - 1))

                # gamma = cm/rs, u = (1-cm)/LQ, u2 = (1-qm)/LC packed as bf16
                # columns of one tile; one PE transpose + sbuf->sbuf DMA puts
                # every row vector on partition 0 (matmul base-partition rule).
                rsi_t = sml.tile([128, NLC], F32, tag="rsi_t")
                nc.vector.reciprocal(rsi_t, rs_ps)
                comb_t = sml.tile([128, 2 * NLC + NJC], BF16, tag="comb_t")
                nc.vector.tensor_mul(comb_t[:, 0:NLC], cm_f, rsi_t)
                nc.vector.tensor_scalar(
                    comb_t[:, NLC:2 * NLC], cm_f, -1.0 / LQ, 1.0 / LQ, MUL, ADD)

                # qsum/csum as bf16 rows (1,128) via (128,1) PE transposes
                qsum_b = sml.tile([128, 1], BF16, tag="qsum_b")
                nc.vector.tensor_copy(qsum_b, qsum_t)
                tp3_ps = ps.tile([1, 128], BF16, tag="misc", bufs=2, name="tp3_ps")
                nc.tensor.transpose(tp3_ps, qsum_b, ident)
                qsumT = sml.tile([1, 128], BF16, tag="qsumT")
                nc.vector.tensor_copy(qsumT, tp3_ps)
                csum_b = sml.tile([128, 1], BF16, tag="csum_b")
                nc.vector.tensor_copy(csum_b, csum_t)
                tp4_ps = ps.tile([1, 128], BF16, tag="misc", bufs=2, name="tp4_ps")
                nc.tensor.transpose(tp4_ps, csum_b, ident)
                csumT = sml.tile([1, 128], BF16, tag="csumT")
                nc.vector.tensor_copy(csumT, tp4_ps)

                # ---- F = exp(cq + r_i), (Lc-part, Lq-free) bf16 ----
                f_t = big.tile([128, NLC, LQ], BF16, tag="f_t")
                for ii in range(NLC):
                    s_ps = ps.tile([128, LQ + 1], F32, tag="sp", bufs=2, name="s_ps")
                    nc.tensor.matmul(
                        s_ps, cb_t[:, ii * 128:(ii + 1) * 128], qw_t,
                        start=True, stop=True)
                    nc.vector.tensor_copy(r_sb[:, ii:ii + 1], s_ps[:, LQ:LQ + 1])
                    nc.scalar.activation(f_t[:, ii, :], s_ps[:, 0:LQ], EXP,
                                         bias=r_sb[:, ii:ii + 1])

                nc.vector.tensor_scalar(
                    comb_t[:, 2 * NLC:2 * NLC + NJC], qm_f,
                    -1.0 / LC, 1.0 / LC, MUL, ADD)
                # transpose packed rows, flatten onto partition 0 via DMA
                tp_ps = ps.tile([2 * NLC + NJC, 128], BF16, tag="misc", bufs=2,
                                name="tp_ps")
                nc.tensor.transpose(tp_ps, comb_t, ident)
                combT = sml.tile([2 * NLC + NJC, 128], BF16, tag="combT")
                nc.vector.tensor_copy(combT, tp_ps)
                rows_t = sml.tile([1, (2 * NLC + NJC) * 128], BF16, tag="rows_t")
                nc.sync.dma_start(
                    out=rows_t.rearrange("o (r x) -> o r x", x=128), in_=combT)

                # ---- ct (bf16, (Lc-part, d+1)) via one xbar DMA transpose;
                # the ones column makes the s2tc matmul emit colsum cs_j free.
                # inner stride padded to 144 elems (288B) so each chunk's
                # xbar write target stays 32-byte aligned
                ct_t = big.tile([128, NLC, 144], BF16, tag="ct_t")
                nc.vector.memset(ct_t[:, :, 128:129], 1.0)
                nc.sync.dma_start(out=ct_t[:, :, 0:128], in_=cb_t, transpose=True)

                # qT (Lq-part, d) bf16
                qT_t = sml.tile([128, NJC, 128], BF16, tag="qT_t")
                for jc in range(NJC):
                    qtp = ps.tile([128, 128], BF16, tag="misc", bufs=2, name="qtp")
                    nc.tensor.transpose(qtp, qb_t[:, jc * 128:(jc + 1) * 128], ident)
                    nc.vector.tensor_copy(qT_t[:, jc, :], qtp)

                # ---- s2tc = fixup(s2^T @ ct), (Lq-part, d) bf16 ----
                s2tc_t = sml.tile([128, NJC, 128], BF16, tag="s2tc_t")
                for jj in range(NJC):
                    ftc_ps = ps.tile([128, 129], F32, tag="misc", bufs=2, name="ftc_ps")
                    for ii in range(NLC):
                        nc.tensor.matmul(
                            ftc_ps, f_t[:, ii, jj * 128:(jj + 1) * 128],
                            ct_t[:, ii, 0:129], start=(ii == 0), stop=(ii == NLC - 1))
                    csi_t = sml.tile([128, 1], F32, tag="csi_t")
                    nc.vector.reciprocal(csi_t, ftc_ps[:, 128:129])
                    al2_t = sml.tile([128, 1], F32, tag="al2_t")
                    nc.vector.tensor_mul(al2_t, qm_f[:, jj:jj + 1], csi_t)
                    t2_ps = ps.tile([128, 128], F32, tag="misc", bufs=2, name="t2_ps")
                    nc.tensor.matmul(
                        t2_ps, rows_t[:, (2 * NLC + jj) * 128:(2 * NLC + jj + 1) * 128],
                        csumT, start=True, stop=True)
                    t2_sb = sml.tile([128, 128], BF16, tag="t2_sb")
                    nc.vector.tensor_copy(t2_sb, t2_ps)
                    nc.vector.scalar_tensor_tensor(
                        out=s2tc_t[:, jj, :], in0=ftc_ps[:, 0:128], scalar=al2_t,
                        in1=t2_sb, op0=MUL, op1=ADD)

                # s2sum row (1,128) bf16
                s2s_ps = ps.tile([1, 128], F32, tag="misc", bufs=2, name="s2s_ps")
                for jj in range(NJC):
                    nc.tensor.matmul(s2s_ps, ones_col_b, s2tc_t[:, jj, :],
                                     start=(jj == 0), stop=(jj == NJC - 1))
                s2sumT = sml.tile([1, 128], BF16, tag="s2sumT")
                nc.vector.tensor_copy(s2sumT, s2s_ps)

                # ---- per-tile: Gb bcast, s1, A/B matmuls, outputs ----
                a_sb = big.tile([128, LC], F32, tag="a_sb")
                blk3 = big.tile([128, LC], F32, tag="blk3")
                blk4 = big.tile([128, LC], F32, tag="blk4")
                s1_t = big.tile([128, NJC, LC], BF16, tag="s1_t")
                for n in range(NT):
                    sl = slice(n * 512, (n + 1) * 512)
                    gb_ps = ps.tile([128, 512], F32, tag="gb", bufs=1, name="gb_ps")
                    nc.tensor.matmul(
                        gb_ps, ones_row_b,
                        rows_t[:, n * 512:(n + 1) * 512], start=True, stop=True)
                    for jc in range(NJC):
                        nc.vector.tensor_mul(s1_t[:, jc, sl], e1_t[:, jc, sl], gb_ps)

                    a_ps = ps.tile([128, 512], F32, tag="a", bufs=2, name="a_ps")
                    for jc in range(NJC):
                        nc.tensor.matmul(a_ps, qT_t[:, jc, :], s1_t[:, jc, sl],
                                         start=(jc == 0), stop=False)
                    nc.tensor.matmul(
                        a_ps, qsumT,
                        rows_t[:, NLC * 128 + n * 512:NLC * 128 + (n + 1) * 512],
                        start=False, stop=True)
                    nc.vector.tensor_copy(a_sb[:, sl], a_ps)

                    b_ps = ps.tile([128, 512], F32, tag="b", bufs=1, name="b_ps")
                    for jc in range(NJC):
                        nc.tensor.matmul(b_ps, s2tc_t[:, jc, :], s1_t[:, jc, sl],
                                         start=(jc == 0), stop=False)
                    nc.tensor.matmul(
                        b_ps, s2sumT,
                        rows_t[:, NLC * 128 + n * 512:NLC * 128 + (n + 1) * 512],
                        start=False, stop=True)
                    nc.vector.tensor_mul(blk4[:, sl], c_t[:, sl], b_ps)
                    nc.gpsimd.tensor_tensor(blk3[:, sl], c_t[:, sl], a_sb[:, sl], MUL)

                nc.sync.dma_start(out=out_d[b, 0:128, :], in_=c_t)
                nc.sync.dma_start(out=out_d[b, 128:256, :], in_=a_sb)
                nc.sync.dma_start(out=out_d[b, 256:384, :], in_=blk3)
                nc.sync.dma_start(out=out_d[b, 384:512, :], in_=blk4)

    return nc


_CACHE = {}


def kernel(c, c_mask, q, q_mask, w, b=None, **_ignored):
    c = np.ascontiguousarray(np.asarray(c, dtype=np.float32))
    q = np.ascontiguousarray(np.asarray(q, dtype=np.float32))
    c_mask = np.ascontiguousarray(np.asarray(c_mask, dtype=np.int32))
    q_mask = np.ascontiguousarray(np.asarray(q_mask, dtype=np.int32))
    w = np.ascontiguousarray(np.asarray(w, dtype=np.float32))

    if "nc" not in _CACHE:
        nc = build_nc()
        nc.compile()
        _CACHE["nc"] = nc
    nc = _CACHE["nc"]

    ident = np.eye(128, dtype=ml_dtypes.bfloat16)
    in_maps = []
    for k in range(NCORES):
        s = slice(k * BPC, (k + 1) * BPC)
        in_maps.append({
            "c": np.ascontiguousarray(c[s]),
            "c_mask": np.ascontiguousarray(c_mask[s]),
            "q": np.ascontiguousarray(q[s]),
            "q_mask": np.ascontiguousarray(q_mask[s]),
            "w": w,
            "ident": ident,
        })
    _CACHE["last_in_maps"] = in_maps
    res = run_bass_kernel_spmd(nc, in_maps, list(range(NCORES)),
                               trace=_CACHE.get("trace", False))
    _CACHE["last_exec_ns"] = res.exec_time_ns
    _CACHE["last_results"] = res
    out = np.concatenate([res.results[k]["out"] for k in range(NCORES)], axis=0)
    return out


def last_exec_ns():
    return _CACHE.get("last_exec_ns")
